# revision 5
# baseline (speedup 1.0000x reference)
"""AtomEmbedding kernel for 8 TRN2 NeuronCores.

Reference semantics: rank-remap of atom types through the sorted unique
values present in the batch, then embedding lookup:
    uniq = unique(atom_types)  (padded sorted)
    out[b, a] = embedding[searchsorted(uniq, atom_types[b, a])]

Device strategy (data-parallel over n_batch): the lookup is computed as a
one-hot matmul on the TensorEngine. v3 pipeline per core:

  stage : idx feed (50 host-made copies, int8) DMA'd once into SBUF in 3
          pieces (3.7 MB HBM read instead of 7.4 MB of broadcast re-reads).
  per pair p of super-batches (8192 atoms = 16 chunks of 512):
    SYNC : 2 SBUF->SBUF DMAs [50, 8192] replicate the idx bytes onto
           100 partitions (8 KB descriptors, HWDGE ring, ~0.65 us issue)
  per chunk k (512 atoms):
    DVE  : oh[t, q] = (rep == iota_t)      bf16 one-hot
    PE   : out[q, d] = oh[:, s*128:].T @ tbl_bf16[100, 64] -> PSUM
  per super-batch sb (8 chunks):
    ACT  : 2 copies [128, 1024] PSUM->SBUF
    SYNC : one 1 MB DMA SBUF->DRAM.

DMA-completion semaphores are PER RING SLOT: increments from distinct
in-flight DMAs interleave across the 16 SDMA engines, so a single
cumulative counter can reach its threshold before any one DMA finished.

Raw-bass engine blocks with standalone wait_ge (the neuronxcc walrus in
this toolchain cannot encode multi-wait sync on one instruction).

Self-contained: shapes hardcoded, no sibling imports.
"""

import sys

if "/opt/trn_rl_repo" not in sys.path:
    sys.path.insert(0, "/opt/trn_rl_repo")

import numpy as np

N_BATCH = 9000
ATOMS_PER_MOL = 64
EMBED_DIM = 64
NUM_TYPES = 100
N_CORES = 8

ROWS_PER_CORE = N_BATCH * ATOMS_PER_MOL // N_CORES  # 72000
T_SLOTS = 576  # padded slots per partition
PAD_ROWS = 128 * T_SLOTS  # 73728
N_CHUNKS = T_SLOTS // 4  # 144 chunks of 512 atoms
N_SB = N_CHUNKS // 8  # 18 super-batches of 4096 atoms
N_PAIR = N_SB // 2  # 9 replicate pairs of 8192 atoms
N_COPIES = 50  # idx feed copies staged in SBUF (fanout 2 per copy)
N_STAGE = 3  # staging pieces (3 pairs each)

_CACHE = {}


def _build_graph():
    import concourse.bass as bass
    import concourse.mybir as mybir

    f32 = mybir.dt.float32
    bf16 = mybir.dt.bfloat16
    AF = mybir.ActivationFunctionType
    OP = mybir.AluOpType

    nc = bass.Bass()

    i8 = mybir.dt.int8
    idx_d = nc.declare_dram_parameter(
        "idx", [N_COPIES, N_CHUNKS * 512], i8, isOutput=False
    )
    iota_d = nc.declare_dram_parameter("iota", [128, 1], f32, isOutput=False)
    tbl_d = nc.declare_dram_parameter("tbl", [128, 64], bf16, isOutput=False)
    out_d = nc.declare_dram_parameter("out", [PAD_ROWS, EMBED_DIM], f32, isOutput=True)

    from contextlib import ExitStack

    PIECE = N_CHUNKS * 512 // N_STAGE  # staging piece width (bytes per copy row)
    PR_W = 8192  # pair atoms

    with ExitStack() as stack:
        iota_sb = stack.enter_context(nc.sbuf_tensor("iota_sb", [128, 1], f32))
        tbl_sb = stack.enter_context(nc.sbuf_tensor("tbl_sb", [128, 64], bf16))
        idx_sb = stack.enter_context(
            nc.sbuf_tensor("idx_sb", [N_COPIES, N_CHUNKS * 512], i8)
        )
        rep_sb = stack.enter_context(nc.sbuf_tensor("rep_sb", [100, 2 * PR_W], i8))
        oh_sb = stack.enter_context(nc.sbuf_tensor("oh_sb", [100, 4 * 512], bf16))
        outb_sb = stack.enter_context(nc.sbuf_tensor("outb_sb", [128, 3 * 2048], f32))
        pout = [
            stack.enter_context(nc.psum_tensor(f"pout{_i}", [128, 2048], f32))
            for _i in range(2)
        ]
        in1 = stack.enter_context(nc.semaphore("in1"))
        in2 = stack.enter_context(nc.semaphore("in2"))
        stg = [stack.enter_context(nc.semaphore(f"stg{i}")) for i in range(N_STAGE)]
        rep_p = [stack.enter_context(nc.semaphore(f"rep_p{i}")) for i in range(2)]
        oh_rdy = stack.enter_context(nc.semaphore("oh_rdy"))
        pout_rdy = stack.enter_context(nc.semaphore("pout_rdy"))
        wbA = stack.enter_context(nc.semaphore("wbA"))
        wbd = [stack.enter_context(nc.semaphore(f"wbd{i}")) for i in range(3)]
        block = stack.enter_context(nc.Block())
        iota_ap = iota_sb[:100, 0:1]
        out_flat = out_d[:].rearrange("(p t) d -> p (t d)", p=128)

        def ohv(k):
            return oh_sb[:, (k % 4) * 512 : (k % 4) * 512 + 512]

        def repv(k):
            p = k // 16
            base = (p % 2) * PR_W + (k % 16) * 512
            return rep_sb[:, base : base + 512]

        def issue_rep(eng, p):
            # replicate idx feed cols [p*8192, (p+1)*8192) onto 100
            # partitions: 2 SBUF->SBUF copies of [50, 8192]
            eng.wait_ge(stg[(2 * p) // (2 * N_PAIR // N_STAGE)], 16)
            if p >= 2:
                eng.wait_ge(oh_rdy, 16 * (p - 1))  # DVE consumed pair p-2
            for c in range(2):
                eng.dma_start(
                    out=rep_sb[
                        c * N_COPIES : (c + 1) * N_COPIES,
                        (p % 2) * PR_W : (p % 2) * PR_W + PR_W,
                    ],
                    in_=idx_sb[:, p * PR_W : (p + 1) * PR_W],
                ).then_inc(rep_p[p % 2], 16)

        @block.scalar
        def _(act):
            # staging pieces 1..: own HWDGE ring, drains while sync's ring
            # serves piece 0 + the first replicate pair
            for i in range(1, N_STAGE):
                act.dma_start(
                    out=idx_sb[:, i * PIECE : (i + 1) * PIECE],
                    in_=idx_d[:, i * PIECE : (i + 1) * PIECE],
                ).then_inc(stg[i], 16)
            for sb in range(N_SB):
                # outb slot free (out DMA of sb-3 done)
                if sb >= 3:
                    act.wait_ge(wbd[sb % 3], 16 * (sb // 3))
                act.wait_ge(pout_rdy, 8 * sb + 4)
                act.activation(
                    out=outb_sb[:, (sb % 3) * 2048 : (sb % 3) * 2048 + 1024],
                    in_=pout[sb % 2][:, :1024],
                    func=AF.Copy,
                ).then_inc(wbA, 1)
                act.wait_ge(pout_rdy, 8 * (sb + 1))
                act.activation(
                    out=outb_sb[:, (sb % 3) * 2048 + 1024 : (sb % 3) * 2048 + 2048],
                    in_=pout[sb % 2][:, 1024:],
                    func=AF.Copy,
                ).then_inc(wbA, 1)

        @block.vector
        def _(dve):
            dve.wait_ge(in1, 16)
            for k in range(N_CHUNKS):
                p = k // 16
                if k % 16 == 0:
                    dve.wait_ge(rep_p[p % 2], 32 * (p // 2 + 1))
                if k >= 4:
                    dve.wait_ge(pout_rdy, k - 3)  # PE freed oh[k%4]
                dve.tensor_scalar(
                    out=ohv(k),
                    in0=repv(k),
                    scalar1=iota_ap,
                    scalar2=None,
                    op0=OP.is_equal,
                ).then_inc(oh_rdy, 1)

        @block.tensor
        def _(te):
            te.wait_ge(in2, 16)
            for k in range(N_CHUNKS):
                sb = k // 8
                if k % 8 == 0 and sb >= 2:
                    te.wait_ge(wbA, 2 * (sb - 1))  # ACT copied both halves of sb-2
                te.wait_ge(oh_rdy, k + 1)
                for s in range(4):
                    mm = te.matmul(
                        out=pout[sb % 2][
                            :, (k % 8) * 256 + s * 64 : (k % 8) * 256 + (s + 1) * 64
                        ],
                        lhsT=ohv(k)[:, s * 128 : (s + 1) * 128],
                        rhs=tbl_sb[:100, :],
                        start=True,
                        stop=True,
                    )
                    if s == 3:
                        mm.then_inc(pout_rdy, 1)

        @block.sync
        def _(sync):
            sync.dma_start(out=iota_sb[:], in_=iota_d[:]).then_inc(in1, 16)
            sync.dma_start(out=tbl_sb[:], in_=tbl_d[:]).then_inc(in2, 16)
            sync.dma_start(
                out=idx_sb[:, 0:PIECE],
                in_=idx_d[:, 0:PIECE],
            ).then_inc(stg[0], 16)
            issue_rep(sync, 0)
            issue_rep(sync, 1)
            for sb in range(N_SB):
                if sb % 2 == 0 and sb // 2 + 2 < N_PAIR:
                    issue_rep(sync, sb // 2 + 2)
                sync.wait_ge(wbA, 2 * (sb + 1))
                if sb >= 3:
                    sync.wait_ge(wbd[sb % 3], 16 * (sb // 3))
                sync.dma_start(
                    out=out_flat[:, sb * 2048 : (sb + 1) * 2048],
                    in_=outb_sb[:, (sb % 3) * 2048 : (sb % 3) * 2048 + 2048],
                ).then_inc(wbd[sb % 3], 16)
            for i in range(3):
                sync.wait_ge(wbd[i], 16 * (N_SB // 3))

    return nc


def _prep_in_maps(atom_types, embedding):
    import ml_dtypes

    at = np.asarray(atom_types).astype(np.int32).reshape(-1)
    emb = np.asarray(embedding).astype(np.float32)

    # rank-remap: table2[x] = embedding[rank(x)] where rank(x) counts the
    # distinct values < x present anywhere in the batch (identity when all
    # NUM_TYPES values appear).
    present = np.zeros(NUM_TYPES, dtype=bool)
    present[at] = True
    rank = np.cumsum(present) - present
    table2 = emb[np.minimum(rank, NUM_TYPES - 1)].astype(np.float32)
    table2[~present] = 0.0

    iota_in = np.arange(128, dtype=np.float32).reshape(128, 1)
    tbl_in = np.zeros((128, 64), np.float32)
    tbl_in[:100] = table2

    in_maps = []
    for c in range(N_CORES):
        shard = at[c * ROWS_PER_CORE : (c + 1) * ROWS_PER_CORE]
        sp = np.concatenate(
            [shard, np.full(PAD_ROWS - ROWS_PER_CORE, shard[0], np.int32)]
        )
        # atom (p, t) = sp[p*T + t]; chunk k covers slots 4k..4k+3;
        # within-chunk position q = s*128 + p.
        grid = sp.reshape(128, T_SLOTS)  # [p, t]
        feed = grid.reshape(128, N_CHUNKS, 4).transpose(1, 2, 0)  # [k, s, p]
        in_maps.append(
            {
                "idx": np.tile(feed.reshape(1, -1).astype(np.int8), (N_COPIES, 1)),
                "iota": iota_in,
                "tbl": tbl_in.astype(ml_dtypes.bfloat16),
            }
        )
    return in_maps


def run(atom_types, embedding, trace=False):
    from concourse.bass_utils import run_bass_kernel_spmd

    if "nc" not in _CACHE:
        _CACHE["nc"] = _build_graph()
    nc = _CACHE["nc"]

    in_maps = _prep_in_maps(atom_types, embedding)
    res = run_bass_kernel_spmd(
        nc, in_maps, core_ids=list(range(N_CORES)), trace=trace
    )
    shards = [r["out"][:ROWS_PER_CORE] for r in res.results]
    full = np.concatenate(shards, axis=0).reshape(N_BATCH, ATOMS_PER_MOL, EMBED_DIM)
    return np.ascontiguousarray(full, dtype=np.float32), res


def kernel(atom_types, embedding):
    out, _ = run(atom_types, embedding, trace=False)
    return out


# revision 6
# speedup vs baseline: 1.4264x; 1.4264x over previous
"""AtomEmbedding kernel for 8 TRN2 NeuronCores.

Reference semantics: rank-remap of atom types through the sorted unique
values present in the batch, then embedding lookup:
    uniq = unique(atom_types)  (padded sorted)
    out[b, a] = embedding[searchsorted(uniq, atom_types[b, a])]

Device strategy (data-parallel over n_batch): the lookup is computed as a
one-hot matmul on the TensorEngine. v4 pipeline per core:

  per pair p of super-batches (8192 atoms = 16 chunks of 512):
    ACT  : one broadcast DMA [100, 8192] HBM->SBUF replicating the idx
           bytes (10 DRAM copies x 10-way fanout, 8 KB descriptors,
           HWDGE ring, one ~0.7 us issue per 800 KB)
  per chunk k (512 atoms):
    DVE  : oh[t, q] = (rep == iota_t)      bf16 one-hot
    PE   : out[q, d] = oh[:, s*128:].T @ tbl_bf16[100, 64] -> PSUM
  per super-batch sb (8 chunks):
    ACT  : 2 copies [128, 1024] PSUM->SBUF
    SYNC : one 1 MB DMA SBUF->DRAM.

DMA-completion semaphores are PER RING SLOT: increments from distinct
in-flight DMAs interleave across the 16 SDMA engines, so a single
cumulative counter can reach its threshold before any one DMA finished.

Raw-bass engine blocks with standalone wait_ge (the neuronxcc walrus in
this toolchain cannot encode multi-wait sync on one instruction).

Self-contained: shapes hardcoded, no sibling imports.
"""

import sys

if "/opt/trn_rl_repo" not in sys.path:
    sys.path.insert(0, "/opt/trn_rl_repo")

import numpy as np

N_BATCH = 9000
ATOMS_PER_MOL = 64
EMBED_DIM = 64
NUM_TYPES = 100
N_CORES = 8

ROWS_PER_CORE = N_BATCH * ATOMS_PER_MOL // N_CORES  # 72000
T_SLOTS = 576  # padded slots per partition
PAD_ROWS = 128 * T_SLOTS  # 73728
N_CHUNKS = T_SLOTS // 4  # 144 chunks of 512 atoms
N_SB = N_CHUNKS // 8  # 18 super-batches of 4096 atoms
N_PAIR = N_SB // 2  # 9 replicate pairs of 8192 atoms
N_COPIES = 10  # idx feed copies in DRAM (fanout 10 per copy)

_CACHE = {}


def _build_graph():
    import concourse.bass as bass
    import concourse.mybir as mybir

    f32 = mybir.dt.float32
    bf16 = mybir.dt.bfloat16
    AF = mybir.ActivationFunctionType
    OP = mybir.AluOpType

    nc = bass.Bass()

    i8 = mybir.dt.int8
    idx_d = nc.declare_dram_parameter(
        "idx", [N_COPIES, N_CHUNKS * 512], i8, isOutput=False
    )
    iota_d = nc.declare_dram_parameter("iota", [128, 1], f32, isOutput=False)
    tbl_d = nc.declare_dram_parameter("tbl", [128, 64], bf16, isOutput=False)
    out_d = nc.declare_dram_parameter("out", [PAD_ROWS, EMBED_DIM], f32, isOutput=True)

    from contextlib import ExitStack

    PR_W = 8192  # pair atoms

    with ExitStack() as stack:
        iota_sb = stack.enter_context(nc.sbuf_tensor("iota_sb", [128, 1], f32))
        tbl_sb = stack.enter_context(nc.sbuf_tensor("tbl_sb", [128, 64], bf16))
        rep_sb = stack.enter_context(nc.sbuf_tensor("rep_sb", [100, 2 * PR_W], i8))
        oh_sb = stack.enter_context(nc.sbuf_tensor("oh_sb", [100, 4 * 512], bf16))
        outb_sb = stack.enter_context(nc.sbuf_tensor("outb_sb", [128, 3 * 2048], f32))
        pout = [
            stack.enter_context(nc.psum_tensor(f"pout{_i}", [128, 2048], f32))
            for _i in range(2)
        ]
        in1 = stack.enter_context(nc.semaphore("in1"))
        in2 = stack.enter_context(nc.semaphore("in2"))
        rep_p = [stack.enter_context(nc.semaphore(f"rep_p{i}")) for i in range(2)]
        oh_rdy = stack.enter_context(nc.semaphore("oh_rdy"))
        pout_rdy = stack.enter_context(nc.semaphore("pout_rdy"))
        wbA = stack.enter_context(nc.semaphore("wbA"))
        wbd = [stack.enter_context(nc.semaphore(f"wbd{i}")) for i in range(3)]
        block = stack.enter_context(nc.Block())
        iota_ap = iota_sb[:100, 0:1]
        out_flat = out_d[:].rearrange("(p t) d -> p (t d)", p=128)

        def ohv(k):
            return oh_sb[:, (k % 4) * 512 : (k % 4) * 512 + 512]

        def repv(k):
            p = k // 16
            base = (p % 2) * PR_W + (k % 16) * 512
            return rep_sb[:, base : base + 512]

        def issue_rep(eng, p):
            # replicate idx feed cols [p*8192, (p+1)*8192) onto 100
            # partitions: 10 DRAM copies x 10-way 0-stride fanout
            if p >= 2:
                eng.wait_ge(oh_rdy, 16 * (p - 1))  # DVE consumed pair p-2
            src = bass.AP(
                idx_d[:].tensor,
                p * PR_W,
                [[N_CHUNKS * 512, N_COPIES], [0, 10], [1, PR_W]],
            )
            eng.dma_start(
                out=rep_sb[:, (p % 2) * PR_W : (p % 2) * PR_W + PR_W],
                in_=src,
            ).then_inc(rep_p[p % 2], 16)

        @block.scalar
        def _(act):
            issue_rep(act, 0)
            issue_rep(act, 1)
            for sb in range(N_SB):
                if sb % 2 == 0 and sb // 2 + 2 < N_PAIR:
                    issue_rep(act, sb // 2 + 2)
                # outb slot free (out DMA of sb-3 done)
                if sb >= 3:
                    act.wait_ge(wbd[sb % 3], 16 * (sb // 3))
                act.wait_ge(pout_rdy, 8 * sb + 4)
                act.activation(
                    out=outb_sb[:, (sb % 3) * 2048 : (sb % 3) * 2048 + 1024],
                    in_=pout[sb % 2][:, :1024],
                    func=AF.Copy,
                ).then_inc(wbA, 1)
                act.wait_ge(pout_rdy, 8 * (sb + 1))
                act.activation(
                    out=outb_sb[:, (sb % 3) * 2048 + 1024 : (sb % 3) * 2048 + 2048],
                    in_=pout[sb % 2][:, 1024:],
                    func=AF.Copy,
                ).then_inc(wbA, 1)

        @block.vector
        def _(dve):
            dve.wait_ge(in1, 16)
            for k in range(N_CHUNKS):
                p = k // 16
                if k % 16 == 0:
                    dve.wait_ge(rep_p[p % 2], 16 * (p // 2 + 1))
                if k >= 4:
                    dve.wait_ge(pout_rdy, k - 3)  # PE freed oh[k%4]
                dve.tensor_scalar(
                    out=ohv(k),
                    in0=repv(k),
                    scalar1=iota_ap,
                    scalar2=None,
                    op0=OP.is_equal,
                ).then_inc(oh_rdy, 1)

        @block.tensor
        def _(te):
            te.wait_ge(in2, 16)
            for k in range(N_CHUNKS):
                sb = k // 8
                if k % 8 == 0 and sb >= 2:
                    te.wait_ge(wbA, 2 * (sb - 1))  # ACT copied both halves of sb-2
                te.wait_ge(oh_rdy, k + 1)
                for s in range(4):
                    mm = te.matmul(
                        out=pout[sb % 2][
                            :, (k % 8) * 256 + s * 64 : (k % 8) * 256 + (s + 1) * 64
                        ],
                        lhsT=ohv(k)[:, s * 128 : (s + 1) * 128],
                        rhs=tbl_sb[:100, :],
                        start=True,
                        stop=True,
                    )
                    if s == 3:
                        mm.then_inc(pout_rdy, 1)

        @block.sync
        def _(sync):
            sync.dma_start(out=iota_sb[:], in_=iota_d[:]).then_inc(in1, 16)
            sync.dma_start(out=tbl_sb[:], in_=tbl_d[:]).then_inc(in2, 16)
            for sb in range(N_SB):
                sync.wait_ge(wbA, 2 * (sb + 1))
                if sb >= 3:
                    sync.wait_ge(wbd[sb % 3], 16 * (sb // 3))
                sync.dma_start(
                    out=out_flat[:, sb * 2048 : (sb + 1) * 2048],
                    in_=outb_sb[:, (sb % 3) * 2048 : (sb % 3) * 2048 + 2048],
                ).then_inc(wbd[sb % 3], 16)
            for i in range(3):
                sync.wait_ge(wbd[i], 16 * (N_SB // 3))

    return nc


def _prep_in_maps(atom_types, embedding):
    import ml_dtypes

    at = np.asarray(atom_types).astype(np.int32).reshape(-1)
    emb = np.asarray(embedding).astype(np.float32)

    # rank-remap: table2[x] = embedding[rank(x)] where rank(x) counts the
    # distinct values < x present anywhere in the batch (identity when all
    # NUM_TYPES values appear).
    present = np.zeros(NUM_TYPES, dtype=bool)
    present[at] = True
    rank = np.cumsum(present) - present
    table2 = emb[np.minimum(rank, NUM_TYPES - 1)].astype(np.float32)
    table2[~present] = 0.0

    iota_in = np.arange(128, dtype=np.float32).reshape(128, 1)
    tbl_in = np.zeros((128, 64), np.float32)
    tbl_in[:100] = table2

    in_maps = []
    for c in range(N_CORES):
        shard = at[c * ROWS_PER_CORE : (c + 1) * ROWS_PER_CORE]
        sp = np.concatenate(
            [shard, np.full(PAD_ROWS - ROWS_PER_CORE, shard[0], np.int32)]
        )
        # atom (p, t) = sp[p*T + t]; chunk k covers slots 4k..4k+3;
        # within-chunk position q = s*128 + p.
        grid = sp.reshape(128, T_SLOTS)  # [p, t]
        feed = grid.reshape(128, N_CHUNKS, 4).transpose(1, 2, 0)  # [k, s, p]
        in_maps.append(
            {
                "idx": np.tile(feed.reshape(1, -1).astype(np.int8), (N_COPIES, 1)),
                "iota": iota_in,
                "tbl": tbl_in.astype(ml_dtypes.bfloat16),
            }
        )
    return in_maps


def run(atom_types, embedding, trace=False):
    from concourse.bass_utils import run_bass_kernel_spmd

    if "nc" not in _CACHE:
        _CACHE["nc"] = _build_graph()
    nc = _CACHE["nc"]

    in_maps = _prep_in_maps(atom_types, embedding)
    res = run_bass_kernel_spmd(
        nc, in_maps, core_ids=list(range(N_CORES)), trace=trace
    )
    shards = [r["out"][:ROWS_PER_CORE] for r in res.results]
    full = np.concatenate(shards, axis=0).reshape(N_BATCH, ATOMS_PER_MOL, EMBED_DIM)
    return np.ascontiguousarray(full, dtype=np.float32), res


def kernel(atom_types, embedding):
    out, _ = run(atom_types, embedding, trace=False)
    return out
